# revision 1
# baseline (speedup 1.0000x reference)
"""Trainium2 Bass kernel for the ModelB graph loss.

Strategy (data-parallel over batch, 8 batches per core):
  * node_masks are contiguous prefix masks; each batch's valid region is the
    top-left [n, n] block of its [N, N] matrices.  The host extracts that
    block (gathering first if a mask is ever non-prefix - the sums are node-
    permutation invariant) and packs it, padded, into per-(core, slot) blocks
    whose shapes are shared by all 8 cores, so one SPMD program serves all
    cores.  Batches with n <= 50 are packed two per block at partition
    offsets 0/64; their accumulator columns are split by partition range on
    the host.
  * Pad fills are P=0.5, A=0, R=0: every reduced quantity then has a pad
    contribution of exactly zero except sum(ln(1-P)), sum(ln(P)) and
    sum(P^2).  The ln pads are pad_count*ln(.5) (the device's own value is
    read back through a calibration accumulator lane) and cancel exactly in
    sum(DLT) = sum(LP) - sum(L1P); the P^2 pad is pad_count*0.25.
  * On device each block is a [128, T*N] bf16 SBUF tile (DRAM image is
    pre-interleaved so every partition's span is contiguous).  Per-batch
    masked sums come out as per-partition row-sum columns via fused
    accum_out on ACT/DVE ops; products with no accumulator needs run on
    GPSIMD.  ARI-branch quantities are only computed for slots holding
    n <= 50 batches.  The host reduces the [128, x] stats tensors and
    finishes the scalar arithmetic in float64.
"""

import sys

for _p in ("/opt/trn_rl_repo", "/root/.axon_site/_ro/trn_rl_repo"):
    if _p not in sys.path:
        sys.path.insert(0, _p)

from contextlib import ExitStack

import numpy as np

import concourse.bass as bass  # noqa: F401  (registers engine methods)
import concourse.tile as tile
from concourse import bacc, mybir
from concourse.bass_utils import run_bass_kernel_spmd

N_CORES = 8
B, N, C = 64, 512, 2
N_SLOTS = B // N_CORES  # 8 batches per core
EPS = 1e-8

# stats_v per-slot cols: [AD, SD2, PD, PA, P2, DLT] (large: AD/SD2/DLT)
QV = 6
# stats_a per-slot columns: [LP, L1P, SA, ABS, SD2a] (SD2a: big slots'
# SD^2 row-sum accumulated via ACT Square to offload DVE)
QA = 5

_FT = mybir.dt.float32
_BF = mybir.dt.bfloat16
_AF = mybir.ActivationFunctionType
_OP = mybir.AluOpType

try:
    import ml_dtypes

    _BF_NP = ml_dtypes.bfloat16
except ImportError:  # pragma: no cover
    _BF_NP = None

_build_cache: dict = {}


def _plan(n_list):
    """Choose slot shapes shared by all cores and assign batches to them.

    Rank batches by n descending; rank-group s (8 consecutive ranks) gives
    one batch to every core.  Groups whose members all fit in 64 partitions
    are merged pairwise into "P" slots holding two batches per core at
    partition offsets 0/64.

    Returns (sig, assign) where sig is a tuple of slot descriptors
    ("F", ns, ts, ari) or ("P", f, ari), and assign maps
    (core, slot_index, sub) -> batch index.
    """
    n_arr = np.asarray(n_list)
    order = np.argsort(-n_arr, kind="stable")
    groups = []
    for s in range(N_SLOTS):
        g = order[s * N_CORES : (s + 1) * N_CORES]
        groups.append((int(max(n_arr[b] for b in g)), [int(b) for b in g]))

    sig = []
    assign = {}
    slot = 0
    s = 0
    while s < N_SLOTS:
        ns, g = groups[s]
        if s + 1 < N_SLOTS and ns <= 64 and groups[s + 1][0] <= 64:
            ns2, g2 = groups[s + 1]
            f = max(ns, ns2)
            ari = bool(any(n_arr[b] <= 50 for b in g + g2))
            sig.append(("P", f, ari))
            for c in range(N_CORES):
                assign[(c, slot, 0)] = g[c]
                assign[(c, slot, 1)] = g2[c]
            s += 2
        else:
            ts = max(1, -(-ns // 128))
            ari = bool(any(n_arr[b] <= 50 for b in g))
            # split tall blocks into row-range sub-slots of <=2 segments
            # (finer pipeline grain; all sums split cleanly across rows)
            row0 = 0
            while row0 < ts:
                tseg = ts - row0
                sig.append(("F", ns, tseg, ari, row0))
                for c in range(N_CORES):
                    assign[(c, slot, 0)] = g[c]
                slot += 1
                row0 += tseg
            s += 1
            continue
        slot += 1
    return tuple(sig), assign


def _slot_f(e):
    return e[1] * e[2] if e[0] == "F" else e[1]


def _build(sig):
    nc = bacc.Bacc("TRN2", target_bir_lowering=False, debug=False,
                   num_devices=N_CORES)

    p_in, a_in, r_in = [], [], []
    for s, e in enumerate(sig):
        f = _slot_f(e)
        p_in.append(nc.dram_tensor(f"p{s}", [128, f], _BF,
                                   kind="ExternalInput").ap())
        a_in.append(nc.dram_tensor(f"a{s}", [128, f], _BF,
                                   kind="ExternalInput").ap())
        r_in.append(nc.dram_tensor(f"r{s}", [128, f], _BF,
                                   kind="ExternalInput").ap())
    pc_in = nc.dram_tensor("pc", [128, 64], _FT, kind="ExternalInput").ap()
    pt_in = nc.dram_tensor("pt", [128, 64], _FT, kind="ExternalInput").ap()
    mc_in = nc.dram_tensor("mc", [128, 64], _FT, kind="ExternalInput").ap()
    cal_in = nc.dram_tensor("cal", [1, 2], _BF, kind="ExternalInput").ap()
    nslots = len(sig)
    sv_cols = nslots * QV + 2
    sa_cols = nslots * QA + 1
    sv_out = nc.dram_tensor("sv", [128, sv_cols], _FT,
                            kind="ExternalOutput").ap()
    sa_out = nc.dram_tensor("sa", [128, sa_cols], _FT,
                            kind="ExternalOutput").ap()

    with tile.TileContext(nc) as tc, ExitStack() as ctx:
        pp = ctx.enter_context(tc.tile_pool(name="pp", bufs=4))
        pa = ctx.enter_context(tc.tile_pool(name="pa", bufs=4))
        pr = ctx.enter_context(tc.tile_pool(name="pr", bufs=4))
        pmid = ctx.enter_context(tc.tile_pool(name="pmid", bufs=5))
        pdum = ctx.enter_context(tc.tile_pool(name="pdum", bufs=6))
        pstat = ctx.enter_context(tc.tile_pool(name="pstat", bufs=1))
        psml = ctx.enter_context(tc.tile_pool(name="psml", bufs=1))

        stats_v = pstat.tile([128, sv_cols], _FT, tag="sv")
        stats_a = pstat.tile([128, sa_cols], _FT, tag="sa")
        bm05 = pstat.tile([128, 1], _FT, tag="bm05")
        nc.gpsimd.memset(bm05[:], -0.5)
        bm1 = pstat.tile([128, 1], _FT, tag="bm1")
        nc.gpsimd.memset(bm1[:], -1.0)

        def svc(s, q):
            col = s * QV + q
            return stats_v[:, col : col + 1]

        def sac(s, q):
            col = s * QA + q
            return stats_a[:, col : col + 1]

        # coordinate inputs early on the gpsimd issuer (sync carries the
        # big F-slot loads; compute for these goes mid-stream)
        tpc = psml.tile([128, 64], _FT, tag="tpc")
        nc.gpsimd.dma_start(tpc[:], pc_in[:])
        tpt = psml.tile([128, 64], _FT, tag="tpt")
        nc.gpsimd.dma_start(tpt[:], pt_in[:])
        tmc = psml.tile([128, 64], _FT, tag="tmc")
        nc.gpsimd.dma_start(tmc[:], mc_in[:])
        tcal = psml.tile([1, 2], _BF, tag="tcal")
        nc.gpsimd.dma_start(tcal[:], cal_in[:])

        fslots = [s for s, e in enumerate(sig) if e[0] == "F"]
        pslots = [s for s, e in enumerate(sig) if e[0] == "P"]
        build_order = fslots[:1] + pslots + fslots[1:]
        # dma_starts are completion-serialized on their issuing engine
        # (~600ns+ each); sync carries the large F-slot loads in slot
        # order, gpsimd carries pair-slot and coordinate loads
        def chunked_load(tile_, src_, f, eng):
            eng.dma_start(tile_[:], src_[:])

        mid_emit = (pslots[-1] if pslots else build_order[0])

        def emit_coords():
            # coordinate losses, packed [128, 64] over all 8 local batches
            d = psml.tile([128, 64], _FT, tag="d")
            nc.vector.tensor_sub(d[:], tpc[:], tpt[:])
            dm = psml.tile([128, 64], _FT, tag="dm")
            nc.vector.tensor_mul(dm[:], d[:], tmc[:])
            dsml = psml.tile([128, 64], _FT, tag="dsml")
            nc.vector.scalar_tensor_tensor(
                dsml[:], dm[:], 1.0, dm[:], _OP.mult, _OP.mult,
                accum_out=stats_v[:, nslots * QV : nslots * QV + 1])
            adm = psml.tile([128, 64], _FT, tag="adm")
            nc.scalar.activation(adm[:], dm[:], _AF.Abs)
            hb = psml.tile([128, 64], _FT, tag="hb")
            nc.scalar.activation(hb[:], adm[:], _AF.Relu, bias=bm1[:])
            dsml2 = psml.tile([128, 64], _FT, tag="dsml2")
            nc.vector.scalar_tensor_tensor(
                dsml2[:], hb[:], 1.0, hb[:], _OP.mult, _OP.mult,
                accum_out=stats_v[:, nslots * QV + 1 : nslots * QV + 2])

            # calibration: mirror the L1P op on pad-valued input; the fp32
            # accumulator then reports exactly 2x the per-element pad term.
            dcal = psml.tile([1, 2], _BF, tag="dcal")
            nc.scalar.activation(dcal[:], tcal[:], _AF.Ln, bias=1.0, scale=-1.0,
                                 accum_out=stats_a[0:1, nslots * QA :
                                                   nslots * QA + 1])



        for s in build_order:
            e = sig[s]
            f = _slot_f(e)
            ari = e[-1]
            ldeng = nc.sync if e[0] == "F" else nc.gpsimd
            tp = pp.tile([128, f], _BF, tag="tp")
            chunked_load(tp, p_in[s], f, ldeng)
            ta = pa.tile([128, f], _BF, tag="ta")
            chunked_load(ta, a_in[s], f, ldeng)
            tr = pr.tile([128, f], _BF, tag="tr")
            chunked_load(tr, r_in[s], f, ldeng)

            # ACT: the two logs; their accums give S_LP, S_L1P (and so
            # S_DLT = S_LP - S_L1P on the host, ln(.5) pads cancelling)
            lp = pmid.tile([128, f], _BF, tag="lp")
            nc.scalar.activation(lp[:], tp[:], _AF.Ln,
                                 accum_out=sac(s, 0))
            l1p = pmid.tile([128, f], _BF, tag="l1p")
            nc.scalar.activation(l1p[:], tp[:], _AF.Ln, bias=1.0, scale=-1.0,
                                 accum_out=sac(s, 1))

            # similarity difference: V tensor_tensor is 2x for bf16; use
            # GPSIMD only for smaller slots to keep V for the big ones
            sd = pmid.tile([128, f], _BF, tag="sd")
            if f >= 1500:
                nc.vector.tensor_sub(sd[:], tr[:], ta[:])
            else:
                nc.gpsimd.tensor_sub(sd[:], tr[:], ta[:])

            dlt = pmid.tile([128, f], _BF, tag="dlt")
            nc.vector.tensor_sub(dlt[:], lp[:], l1p[:])

            # DVE: products whose row sums we need
            dv = pdum.tile([128, f], _BF, tag="dv")
            nc.vector.scalar_tensor_tensor(
                dv[:], ta[:], 1.0, dlt[:], _OP.mult, _OP.mult,
                accum_out=svc(s, 0))
            if f >= 1500:
                da4 = pdum.tile([128, f], _BF, tag="da")
                nc.scalar.activation(da4[:], sd[:], _AF.Square,
                                     accum_out=sac(s, 4))
            else:
                dv = pdum.tile([128, f], _BF, tag="dv")
                nc.vector.scalar_tensor_tensor(
                    dv[:], sd[:], 1.0, sd[:], _OP.mult, _OP.mult,
                    accum_out=svc(s, 1))

            if ari:
                # quantities consumed only by the n <= 50 ARI branch
                da2 = pdum.tile([128, f], _BF, tag="da")
                nc.scalar.activation(da2[:], ta[:], _AF.Copy,
                                     accum_out=sac(s, 2))
                da3 = pdum.tile([128, f], _BF, tag="da")
                nc.scalar.activation(da3[:], tp[:], _AF.Abs, bias=bm05[:],
                                     accum_out=sac(s, 3))
                dv = pdum.tile([128, f], _BF, tag="dv")
                nc.vector.scalar_tensor_tensor(
                    dv[:], tp[:], 1.0, dlt[:], _OP.mult, _OP.mult,
                    accum_out=svc(s, 2))
                dv = pdum.tile([128, f], _BF, tag="dv")
                nc.vector.scalar_tensor_tensor(
                    dv[:], tp[:], 1.0, ta[:], _OP.mult, _OP.mult,
                    accum_out=svc(s, 3))
                dv = pdum.tile([128, f], _BF, tag="dv")
                nc.vector.scalar_tensor_tensor(
                    dv[:], tp[:], 1.0, tp[:], _OP.mult, _OP.mult,
                    accum_out=svc(s, 4))

            if s == mid_emit:
                emit_coords()

        nc.sync.dma_start(sv_out[:], stats_v[:])
        nc.sync.dma_start(sa_out[:], stats_a[:])

    nc.compile()
    return nc


def _huber(x):
    ax = np.abs(x)
    return np.where(ax <= 1.0, 0.5 * x * x, ax - 0.5)


def _interleave(block, ts):
    """[ts*128, n] row-major -> [128, ts*n] with per-partition contiguity."""
    if ts == 1:
        return block
    n = block.shape[1]
    return np.ascontiguousarray(
        block.reshape(ts, 128, n).transpose(1, 0, 2).reshape(128, ts * n))


def kernel(predicted_coords, adjacency_matrix, node_counts, raw_similarity,
           temperature, residual_weight, points, adjacency, node_masks,
           _want_results=None):
    masks = np.asarray(node_masks).astype(bool)
    n_list = masks.sum(axis=1).astype(np.int64)
    sig, assign = _plan(n_list)

    if sig not in _build_cache:
        _build_cache[sig] = _build(sig)
    nc = _build_cache[sig]

    p_full = np.asarray(adjacency_matrix, dtype=np.float32)
    a_full = np.asarray(adjacency, dtype=np.float32)
    r_full = np.asarray(raw_similarity, dtype=np.float32)
    pc_full = np.ascontiguousarray(predicted_coords, dtype=np.float32)
    pt_full = np.ascontiguousarray(points, dtype=np.float32)
    m_f32 = masks.astype(np.float32)

    # valid-node index per batch (prefix fast path; gather fallback)
    valid = []
    for b in range(B):
        n = int(n_list[b])
        if masks[b, :n].all():
            valid.append(None)  # prefix: plain slicing
        else:
            valid.append(np.flatnonzero(masks[b]))

    in_maps = []
    for c in range(N_CORES):
        im = {}
        bs = []
        for s, e in enumerate(sig):
            if e[0] == "F":
                _, ns, ts, ari, row0 = e
                b = assign[(c, s, 0)]
                n = int(n_list[b])
                r0 = row0 * 128
                r1 = min(n, r0 + ts * 128)
                nr = max(0, r1 - r0)
                bp = np.full((ts * 128, ns), 0.5, np.float32)
                ba = np.zeros((ts * 128, ns), np.float32)
                br = np.zeros((ts * 128, ns), np.float32)
                if nr > 0:
                    if valid[b] is None:
                        bp[:nr, :n] = p_full[b, r0:r1, :n]
                        ba[:nr, :n] = a_full[b, r0:r1, :n]
                        br[:nr, :n] = r_full[b, r0:r1, :n]
                    else:
                        ix = np.ix_(valid[b][r0:r1], valid[b])
                        bp[:nr, :n] = p_full[b][ix]
                        ba[:nr, :n] = a_full[b][ix]
                        br[:nr, :n] = r_full[b][ix]
                im[f"p{s}"] = _interleave(bp, ts).astype(_BF_NP)
                im[f"a{s}"] = _interleave(ba, ts).astype(_BF_NP)
                im[f"r{s}"] = _interleave(br, ts).astype(_BF_NP)
                if row0 == 0:
                    bs.append(b)
            else:
                _, f, ari = e
                bp = np.full((128, f), 0.5, np.float32)
                ba = np.zeros((128, f), np.float32)
                br = np.zeros((128, f), np.float32)
                for sub, off in ((0, 0), (1, 64)):
                    b = assign[(c, s, sub)]
                    n = int(n_list[b])
                    if valid[b] is None:
                        bp[off:off + n, :n] = p_full[b, :n, :n]
                        ba[off:off + n, :n] = a_full[b, :n, :n]
                        br[off:off + n, :n] = r_full[b, :n, :n]
                    else:
                        ix = np.ix_(valid[b], valid[b])
                        bp[off:off + n, :n] = p_full[b][ix]
                        ba[off:off + n, :n] = a_full[b][ix]
                        br[off:off + n, :n] = r_full[b][ix]
                    bs.append(b)
                im[f"p{s}"] = bp.astype(_BF_NP)
                im[f"a{s}"] = ba.astype(_BF_NP)
                im[f"r{s}"] = br.astype(_BF_NP)
        im["pc"] = pc_full[bs].reshape(128, 64)
        im["pt"] = pt_full[bs].reshape(128, 64)
        im["mc"] = np.repeat(m_f32[bs][:, :, None], C, axis=2).reshape(128, 64)
        im["cal"] = np.array([[0.5, 0.5]], _BF_NP)
        in_maps.append(im)

    res = run_bass_kernel_spmd(nc, in_maps, core_ids=list(range(N_CORES)))
    if _want_results is not None:
        _want_results.append(res)

    # ---- host finalization in float64 ----
    nslots = len(sig)
    sv = [res.results[c]["sv"].astype(np.float64) for c in range(N_CORES)]
    sa = [res.results[c]["sa"].astype(np.float64) for c in range(N_CORES)]
    lnhalf = float(res.results[0]["sa"][0, nslots * QA]) / 2.0

    n_arr = n_list.astype(np.float64)
    cnt_coord = max(float(n_arr.sum()) * C, 1.0)
    cnt2 = max(float((n_arr ** 2).sum()), 1.0)

    s_mse = sum(float(v[:, nslots * QV].sum()) for v in sv)
    s_hsq = sum(float(v[:, nslots * QV + 1].sum()) for v in sv)
    coord_mse = s_mse / cnt_coord
    coord_smooth = (0.5 * s_mse - 0.5 * s_hsq) / cnt_coord
    coord_loss = 0.7 * coord_mse + 0.3 * coord_smooth

    edge_sum = 0.0
    sim_sum = 0.0
    # per-batch accumulation first (a batch may span several row-split
    # slots); the nonlinear ARI math runs once per batch afterwards
    acc = {b: [0.0] * 7 for b in range(B)}  # L1P, DLT, AD, SD2, PD, PA, P2
    acc_a = {b: [0.0] * 2 for b in range(B)}  # SA, ABS
    for c in range(N_CORES):
        for s, e in enumerate(sig):
            if e[0] == "F":
                _, ns, ts, ari, row0 = e
                subs = [(assign[(c, s, 0)], 0, 128, float(ts * 128 * ns),
                         row0, ts)]
            else:
                _, f, ari = e
                subs = [(assign[(c, s, 0)], 0, 64, float(64 * f), 0, 0),
                        (assign[(c, s, 1)], 64, 128, float(64 * f), 0, 0)]
            for b, lo, hi, area, row0, ts_ in subs:
                n = float(n_list[b])
                if e[0] == "F":
                    r0 = row0 * 128
                    nr = max(0.0, min(n, r0 + ts_ * 128) - r0)
                    real = nr * n
                else:
                    real = n * n
                padcnt = area - real
                s_lp = float(sa[c][lo:hi, s * QA + 0].sum())
                s_l1p_raw = float(sa[c][lo:hi, s * QA + 1].sum())
                a = acc[b]
                a[0] += s_l1p_raw - padcnt * lnhalf
                a[1] += s_lp - s_l1p_raw  # ln(.5) pads cancel exactly
                a[2] += float(sv[c][lo:hi, s * QV + 0].sum())
                if e[0] == "F" and _slot_f(e) >= 1500:
                    a[3] += float(sa[c][lo:hi, s * QA + 4].sum())
                else:
                    a[3] += float(sv[c][lo:hi, s * QV + 1].sum())
                if ari:
                    a[4] += float(sv[c][lo:hi, s * QV + 2].sum())
                    a[5] += float(sv[c][lo:hi, s * QV + 3].sum())
                    a[6] += float(sv[c][lo:hi, s * QV + 4].sum()) \
                        - padcnt * 0.25
                    aa = acc_a[b]
                    aa[0] += float(sa[c][lo:hi, s * QA + 2].sum())
                    aa[1] += float(sa[c][lo:hi, s * QA + 3].sum())

    ari_loss = 0.0
    conf_pen = 0.0
    for b in range(B):
        n = float(n_list[b])
        s_l1p, s_dlt, s_ad, s_sd2, s_pd, s_pa, s_p2 = acc[b]
        edge_sum += s_l1p + 0.05 * s_dlt + 0.9 * s_ad
        sim_sum += s_sd2
        if 5.0 < n <= 50.0:
            s_a, s_abs = acc_a[b]
            na = np.sqrt(max(s_p2, 0.0))
            nt = np.sqrt(max(s_a, 0.0))
            cos = s_pa / (max(na, EPS) * max(nt, EPS))
            n2 = max(n * n, 1.0)
            ent = -(s_l1p + s_pd) / n2
            contrast = s_abs / n2
            ari_loss += -cos - 0.2 * contrast
            conf_pen += ent

    edge_loss = -edge_sum / cnt2
    similarity_loss = sim_sum / cnt2

    dc = np.asarray(node_counts, np.float64) - n_arr
    count_loss = float(_huber(dc).mean())
    temp_reg = abs(float(temperature) - 1.0)
    res_reg = abs(float(residual_weight) - 0.5)

    total = (1.0 * coord_loss + 2.0 * edge_loss + 0.1 * count_loss
             + 0.3 * similarity_loss + 0.01 * (temp_reg + res_reg)
             + 1.0 * (ari_loss + 0.1 * conf_pen))
    return np.asarray(total, dtype=np.float32)



# revision 3
# speedup vs baseline: 1.5686x; 1.5686x over previous
"""Trainium2 Bass kernel for the ModelB graph loss.

Strategy: every loss term that touches the [N, N] matrices is a GLOBAL
masked sum (the ARI branch only applies to tiny n <= 50 batches and is
finished on the host from <=2500-element slices).  With binary adjacency
the per-element edge term collapses to

    t_s*ln(p) + (1-t_s)*ln(1-p) = 0.05*ln(X) + 0.95*ln(1-X),  X = |p - a|

so the device only needs three streaming reductions over the packed
valid elements of all batches:

    A1 = sum ln(X)        A2 = sum ln(1-X)        S = sum (r - a)^2

The host packs the valid [n, n] blocks of X = |p-a| and R = r-a into one
flat bf16 stream, split evenly over 8 cores (perfect load balance, no
per-batch slot padding).  On device:

  * ln(X) uses log-pairing: ln(x1*x2*x3*x4) accumulated over quads, so
    ACT (the bottleneck engine) sees F/4 columns instead of F.  The
    pair/quad products run on DVE (bf16 tensor_tensor, 2x mode).
  * ln(1-X) runs directly on ACT with the free affine (scale=-1, bias=1)
    to avoid the catastrophic cancellation a paired (1-x1)(1-x2) would
    hit in bf16.
  * S accumulates via DVE scalar_tensor_tensor.
  * The tiny coordinate loss runs on DVE from a host-fused
    dm = (pred - points) * mask tensor.

Pads (global tail only, < 1024 elements) are X=0.5, R=0; each pad adds
exactly ln(0.5) to A1 and A2 and 0 to S, corrected on the host.
"""

import sys

for _p in ("/opt/trn_rl_repo", "/root/.axon_site/_ro/trn_rl_repo"):
    if _p not in sys.path:
        sys.path.insert(0, _p)

from contextlib import ExitStack

import numpy as np

import concourse.bass as bass  # noqa: F401  (registers engine methods)
import concourse.tile as tile
from concourse import bacc, mybir
from concourse.bass_utils import run_bass_kernel_spmd

N_CORES = 8
B, N, C = 64, 512, 2
G = N_CORES * 128  # global partition count
EPS = 1e-8

_FT = mybir.dt.float32
_BF = mybir.dt.bfloat16
_AF = mybir.ActivationFunctionType
_OP = mybir.AluOpType

try:
    import ml_dtypes

    _BF_NP = ml_dtypes.bfloat16
except ImportError:  # pragma: no cover
    _BF_NP = None

_build_cache: dict = {}

NXC = 2  # x chunks per half (so 2*NXC x DMAs of [128, F/(2*NXC)])
NRC = 2  # r chunks


def _build(F):
    H = F // 2
    Hc = H // NXC
    Hq = Hc // 2
    Rc = F // NRC

    nc = bacc.Bacc("TRN2", target_bir_lowering=False, debug=False,
                   num_devices=N_CORES)

    xa_in = [nc.dram_tensor(f"xa{i}", [128, Hc], _BF,
                            kind="ExternalInput").ap() for i in range(NXC)]
    xb_in = [nc.dram_tensor(f"xb{i}", [128, Hc], _BF,
                            kind="ExternalInput").ap() for i in range(NXC)]
    r_in = [nc.dram_tensor(f"r{i}", [128, Rc], _BF,
                           kind="ExternalInput").ap() for i in range(NRC)]
    dm_in = nc.dram_tensor("dm", [128, 64], _FT, kind="ExternalInput").ap()

    # stats columns: [ln1(xa_i) x NXC | ln1(xb_i) x NXC | lnP_i x NXC |
    #                 r2_j x NRC | mse | hsq]
    KC = 3 * NXC + NRC + 2
    sv_out = nc.dram_tensor("sv", [128, KC], _FT, kind="ExternalOutput").ap()

    with tile.TileContext(nc) as tc, ExitStack() as ctx:
        px = ctx.enter_context(tc.tile_pool(name="px", bufs=2))
        pr = ctx.enter_context(tc.tile_pool(name="pr", bufs=2))
        pmid = ctx.enter_context(tc.tile_pool(name="pmid", bufs=2))
        pdum = ctx.enter_context(tc.tile_pool(name="pdum", bufs=2))
        pstat = ctx.enter_context(tc.tile_pool(name="pstat", bufs=1))

        stats = pstat.tile([128, KC], _FT, tag="sv")

        def svc(q):
            return stats[:, q:q + 1]

        txa = [px.tile([128, Hc], _BF, tag=f"txa{i}", name=f"txa{i}")
               for i in range(NXC)]
        txb = [px.tile([128, Hc], _BF, tag=f"txb{i}", name=f"txb{i}")
               for i in range(NXC)]
        tr = [pr.tile([128, Rc], _BF, tag=f"tr{j}", name=f"tr{j}")
              for j in range(NRC)]
        tdm = pstat.tile([128, 64], _FT, tag="tdm")

        # DMA order on the sync HWDGE ring: x chunks first (ACT needs
        # them earliest), r after, dm on gpsimd.
        for i in range(NXC):
            nc.sync.dma_start(txa[i][:], xa_in[i][:])
            nc.sync.dma_start(txb[i][:], xb_in[i][:])
        for j in range(NRC):
            nc.sync.dma_start(tr[j][:], r_in[j][:])
        nc.gpsimd.dma_start(tdm[:], dm_in[:])

        # ACT: ln(1 - x) over every x chunk (free affine, fp32 internal)
        for i in range(NXC):
            da = pdum.tile([128, Hc], _BF, tag="da")
            nc.scalar.activation(da[:], txa[i][:], _AF.Ln, bias=1.0,
                                 scale=-1.0, accum_out=svc(i))
            db = pdum.tile([128, Hc], _BF, tag="da")
            nc.scalar.activation(db[:], txb[i][:], _AF.Ln, bias=1.0,
                                 scale=-1.0, accum_out=svc(NXC + i))

        # DVE pair/quad products, then ACT ln over F/4 columns
        for i in range(NXC):
            tp = pmid.tile([128, Hc], _BF, tag="tp")
            nc.vector.tensor_mul(tp[:], txa[i][:], txb[i][:])
            tq = pmid.tile([128, Hq], _BF, tag="tq")
            nc.vector.tensor_mul(tq[:], tp[:, :Hq], tp[:, Hq:])
            dq = pdum.tile([128, Hq], _BF, tag="dq")
            nc.scalar.activation(dq[:], tq[:], _AF.Ln,
                                 accum_out=svc(2 * NXC + i))

        # DVE: sum of squares of r chunks
        for j in range(NRC):
            dr = pdum.tile([128, Rc], _BF, tag="dr")
            nc.vector.scalar_tensor_tensor(
                dr[:], tr[j][:], 1.0, tr[j][:], _OP.mult, _OP.mult,
                accum_out=svc(3 * NXC + j))

        # coordinate loss on DVE: mse and squared-huber-excess sums
        dmm = pstat.tile([128, 64], _FT, tag="dmm")
        nc.vector.scalar_tensor_tensor(
            dmm[:], tdm[:], 1.0, tdm[:], _OP.mult, _OP.mult,
            accum_out=svc(3 * NXC + NRC))
        adm = pstat.tile([128, 64], _FT, tag="adm")
        nc.vector.scalar_tensor_tensor(
            adm[:], tdm[:], -1.0, tdm[:], _OP.mult, _OP.max)
        hb = pstat.tile([128, 64], _FT, tag="hb")
        nc.vector.tensor_scalar(hb[:], adm[:], -1.0, 0.0, _OP.add, _OP.max)
        hsq = pstat.tile([128, 64], _FT, tag="hsq")
        nc.vector.scalar_tensor_tensor(
            hsq[:], hb[:], 1.0, hb[:], _OP.mult, _OP.mult,
            accum_out=svc(3 * NXC + NRC + 1))

        nc.gpsimd.dma_start(sv_out[:], stats[:])

    nc.compile()
    return nc


def _huber(x):
    ax = np.abs(x)
    return np.where(ax <= 1.0, 0.5 * x * x, ax - 0.5)


def kernel(predicted_coords, adjacency_matrix, node_counts, raw_similarity,
           temperature, residual_weight, points, adjacency, node_masks,
           _want_results=None):
    masks = np.asarray(node_masks).astype(bool)
    n_list = masks.sum(axis=1).astype(np.int64)

    p_full = np.asarray(adjacency_matrix, dtype=np.float32)
    a_full = np.asarray(adjacency, dtype=np.float32)
    r_full = np.asarray(raw_similarity, dtype=np.float32)
    pc_full = np.ascontiguousarray(predicted_coords, dtype=np.float32)
    pt_full = np.ascontiguousarray(points, dtype=np.float32)

    # valid-node indices (prefix fast path; gather fallback)
    valid = []
    for b in range(B):
        n = int(n_list[b])
        if masks[b, :n].all():
            valid.append(None)
        else:
            valid.append(np.flatnonzero(masks[b]))

    L = int((n_list ** 2).sum())
    chunk = 2 * NXC * np.lcm(NRC, 2)  # F divisible by 2*NXC*2 and NRC
    F = -(-L // (G * chunk)) * chunk
    total = G * F

    if F not in _build_cache:
        _build_cache[F] = _build(F)
    nc = _build_cache[F]

    X_flat = np.empty(total, dtype=_BF_NP)
    R_flat = np.empty(total, dtype=_BF_NP)
    off = 0
    blocks = {}
    for b in range(B):
        n = int(n_list[b])
        if n == 0:
            blocks[b] = None
            continue
        if valid[b] is None:
            ps = p_full[b, :n, :n]
            as_ = a_full[b, :n, :n]
            rs = r_full[b, :n, :n]
        else:
            ix = np.ix_(valid[b], valid[b])
            ps = p_full[b][ix]
            as_ = a_full[b][ix]
            rs = r_full[b][ix]
        blocks[b] = (ps, as_)
        nn = n * n
        X_flat[off:off + nn] = np.abs(ps - as_).ravel().astype(_BF_NP)
        R_flat[off:off + nn] = (rs - as_).ravel().astype(_BF_NP)
        off += nn
    X_flat[L:] = _BF_NP(0.5)
    R_flat[L:] = _BF_NP(0.0)
    X3 = X_flat.reshape(N_CORES, 128, F)
    R3 = R_flat.reshape(N_CORES, 128, F)

    dm_all = ((pc_full - pt_full)
              * masks.astype(np.float32)[:, :, None]).reshape(N_CORES, 128, 64)

    H = F // 2
    Hc = H // NXC
    Rc = F // NRC
    in_maps = []
    for c in range(N_CORES):
        im = {}
        for i in range(NXC):
            im[f"xa{i}"] = np.ascontiguousarray(X3[c, :, i * Hc:(i + 1) * Hc])
            im[f"xb{i}"] = np.ascontiguousarray(
                X3[c, :, H + i * Hc:H + (i + 1) * Hc])
        for j in range(NRC):
            im[f"r{j}"] = np.ascontiguousarray(R3[c, :, j * Rc:(j + 1) * Rc])
        im["dm"] = np.ascontiguousarray(dm_all[c])
        in_maps.append(im)

    res = run_bass_kernel_spmd(nc, in_maps, core_ids=list(range(N_CORES)))
    if _want_results is not None:
        _want_results.append(res)

    # ---- host finalization in float64 ----
    KC = 3 * NXC + NRC + 2
    sv = np.zeros(KC, dtype=np.float64)
    for c in range(N_CORES):
        sv += res.results[c]["sv"].astype(np.float64).sum(axis=0)

    A2 = sv[0:2 * NXC].sum()          # sum ln(1 - X)  (incl pads)
    A1 = sv[2 * NXC:3 * NXC].sum()    # sum ln(X)      (incl pads)
    S = sv[3 * NXC:3 * NXC + NRC].sum()
    s_mse = sv[3 * NXC + NRC]
    s_hsq = sv[3 * NXC + NRC + 1]

    padcnt = float(total - L)
    ln05 = float(np.log(0.5))
    A1 -= padcnt * ln05
    A2 -= padcnt * ln05

    n_arr = n_list.astype(np.float64)
    cnt_coord = max(float(n_arr.sum()) * C, 1.0)
    cnt2 = max(float((n_arr ** 2).sum()), 1.0)

    coord_mse = s_mse / cnt_coord
    coord_smooth = (0.5 * s_mse - 0.5 * s_hsq) / cnt_coord
    coord_loss = 0.7 * coord_mse + 0.3 * coord_smooth

    edge_loss = -(0.05 * A1 + 0.95 * A2) / cnt2
    similarity_loss = S / cnt2

    # ARI branch on host: only 5 < n <= 50 batches, <=2500 elements each
    ari_loss = 0.0
    conf_pen = 0.0
    for b in range(B):
        n = float(n_list[b])
        if not (5.0 < n <= 50.0):
            continue
        ps, as_ = blocks[b]
        ps = ps.astype(np.float64)
        as_ = as_.astype(np.float64)
        dot = float((ps * as_).sum())
        na = np.sqrt(float((ps * ps).sum()))
        nt = np.sqrt(float((as_ * as_).sum()))
        cos = dot / (max(na, EPS) * max(nt, EPS))
        n2 = max(n * n, 1.0)
        ent = -float((ps * np.log(ps + EPS)
                      + (1.0 - ps) * np.log(1.0 - ps + EPS)).sum()) / n2
        contrast = float(np.abs(ps - 0.5).sum()) / n2
        ari_loss += -cos - 0.2 * contrast
        conf_pen += ent

    dc = np.asarray(node_counts, np.float64) - n_arr
    count_loss = float(_huber(dc).mean())
    temp_reg = abs(float(temperature) - 1.0)
    res_reg = abs(float(residual_weight) - 0.5)

    total_loss = (1.0 * coord_loss + 2.0 * edge_loss + 0.1 * count_loss
                  + 0.3 * similarity_loss + 0.01 * (temp_reg + res_reg)
                  + 1.0 * (ari_loss + 0.1 * conf_pen))
    return np.asarray(total_loss, dtype=np.float32)
